# revision 3
# baseline (speedup 1.0000x reference)
"""VQ-codebook 3x3 conv (nn_CConv) on 8 Trainium2 NeuronCores.

Sharding: data-parallel over the batch (16 images -> 2 per core); the small
codebook-derived weights / scales / bias are replicated to every core.
Host-side work is layout only: batch split, reshape/transpose of the index
and scale matrices, and the codebook row gather (pure indexing, no
arithmetic).

Per-core device program (one NEFF, SPMD over 8 cores):
  - weight build (on device): fp16 round-trip of scales (dequant emulation),
    multiply by cut, broadcast-multiply onto the gathered codebook rows
    (shipped k-major so the multiply is one unit-stride op); weights stored
    k-major so each of the 9 taps is a contiguous [128(in), 128(out)] fp16
    stationary block.  Weight DMAs ride the scalar HWDGE queue so they do
    not serialize behind input loads.
  - conv: each image is zero-padded to rows of width 114 in SBUF (borders
    zeroed on-chip); the 3x3 conv is 9 accumulating PE matmuls over shifted
    views of the flattened padded image, fp16 in / fp32 PSUM accumulate.
    PSUM groups are row-aligned (N = 4 rows x 114 = 456) so the junk
    columns w in {112,113} can be dropped during PSUM->SBUF evacuation
    (fused per-partition bias add on the vector engine).  The output slab
    in SBUF is therefore densely packed 112-wide, which makes the output
    DMA fully contiguous on both sides (large packets, ~2.4x the effective
    DMA bandwidth of per-row 448B descriptors).
  - output DMAs alternate between the gpsimd and vector HWDGE queues;
    input loads keep the sync queue to themselves.
  - images are processed in row-slabs (4/24/28 output rows; tiny slabs at
    the kernel's two ends shorten the serial prologue and the final-DMA
    tail); input loads are double-buffered and cast f32->f16 by the scalar
    engine.
  - 8 dummy warm-up matmuls run during the prologue so the PE HAM clock
    gate reaches 2.4 GHz before the real matmuls start.
"""
import sys
import types
from contextlib import ExitStack

import numpy as np

import concourse.tile as tile
from concourse import bacc, mybir


def _ensure_axon_hooks_module():
    """This image's antenv package lacks axon_hooks; bass_utils imports it
    when tracing is requested (e.g. BASS_TRACE=1). Provide a no-op shim."""
    try:
        import antenv

        if "antenv.axon_hooks" not in sys.modules and not hasattr(
            antenv, "axon_hooks"
        ):
            mod = types.ModuleType("antenv.axon_hooks")
            holder = [None]
            mod.set_axon_ntff_profile_hook = lambda h: holder.__setitem__(0, h)
            mod.get_axon_ntff_profile_hook = lambda: holder[0]
            antenv.axon_hooks = mod
            sys.modules["antenv.axon_hooks"] = mod
    except Exception:
        pass


_ensure_axon_hooks_module()

from concourse import bass_utils  # noqa: E402

P = 128
H = W = 112
WP = 114
IMGS = 2
N_CORES = 8

f32 = mybir.dt.float32
f16 = mybir.dt.float16

SLAB_PLAN = {0: [4, 24, 28, 28, 28], 1: [28, 28, 28, 24, 4]}
MAX_SO = 28
GR = 4           # output rows per PSUM group; N = GR*WP = 456
WARM_MMS = 8

_CACHE = {}


def _build():
    nc = bacc.Bacc("TRN2", target_bir_lowering=False, debug=False)

    x_t = nc.dram_tensor("x", [IMGS, P, H, W], f32, kind="ExternalInput")
    scalesT_t = nc.dram_tensor("scalesT", [P, P], f32, kind="ExternalInput")
    cutT_t = nc.dram_tensor("cutT", [P, P], f32, kind="ExternalInput")
    bias_t = nc.dram_tensor("bias", [P, 1], f32, kind="ExternalInput")
    wrawT_t = nc.dram_tensor("wrawT", [P, 9 * P], f32, kind="ExternalInput")
    out_t = nc.dram_tensor("out", [IMGS, P, H, W], f32, kind="ExternalOutput")

    with tile.TileContext(nc) as tc, ExitStack() as ctx:
        wb = ctx.enter_context(tc.tile_pool(name="wb", bufs=1))
        xp = ctx.enter_context(tc.tile_pool(name="xp", bufs=4))
        op = ctx.enter_context(tc.tile_pool(name="op", bufs=4))
        ps = ctx.enter_context(tc.tile_pool(name="ps", bufs=6, space="PSUM"))
        xs = ctx.enter_context(tc.tile_pool(name="xs", bufs=4))

        # PE warmup tile: first gpsimd op so the PE can start ASAP
        wrm = wb.tile([P, 512], f16, tag="warm")
        nc.gpsimd.memset(wrm[:], 0.0)

        # ---- weight DMAs on the scalar queue (parallel to input loads) ----
        w_raw = wb.tile([P, 9 * P], f32, tag="w_raw")
        nc.scalar.dma_start(w_raw[:], wrawT_t.ap())
        sc_in = wb.tile([P, P], f32, tag="sc_in")
        nc.scalar.dma_start(sc_in[:], scalesT_t.ap())
        cut_s = wb.tile([P, P], f32, tag="cut")
        nc.scalar.dma_start(cut_s[:], cutT_t.ap())
        bias_s = wb.tile([P, 1], f32, tag="bias")
        nc.scalar.dma_start(bias_s[:], bias_t.ap())

        # peel slab (0,0) input load so it heads the sync DMA queue
        so0 = SLAB_PLAN[0][0]
        nrows0 = min(H, so0 + 1)
        pre_stage = xs.tile([P, (MAX_SO + 2) * W], f32, tag="xstage")
        nc.sync.dma_start(pre_stage[:, :nrows0 * W], x_t.ap()[0, :, 0:nrows0, :])

        # PE warmup: HAM un-throttles to 2.4 GHz during the prologue
        pw = ps.tile([P, 512], f32, tag="pst")
        for _ in range(WARM_MMS):
            nc.tensor.matmul(pw[:], wrm[:, :P], wrm[:], start=True, stop=True)

        # ---- weight build ----
        sc16 = wb.tile([P, P], f16, tag="sc16")
        nc.vector.tensor_copy(sc16[:], sc_in[:])
        sc = wb.tile([P, P], f32, tag="sc")
        nc.vector.tensor_copy(sc[:], sc16[:])
        scc = wb.tile([P, P], f32, tag="scc")
        nc.vector.tensor_tensor(
            out=scc[:], in0=sc[:], in1=cut_s[:], op=mybir.AluOpType.mult
        )

        # w_mm[i, k, o] = w_raw[i, k, o] * scc[i, o]   (w_raw shipped k-major)
        w_mm = wb.tile([P, 9 * P], f16, tag="w_mm")
        w_raw3 = w_raw[:].rearrange("p (k o) -> p k o", o=P)
        scc3 = scc[:].rearrange("p (one o) -> p one o", one=1).to_broadcast(
            [P, 9, P]
        )
        w_mm3 = w_mm[:].rearrange("p (k o) -> p k o", o=P)
        nc.vector.tensor_tensor(
            out=w_mm3, in0=w_raw3, in1=scc3, op=mybir.AluOpType.mult
        )
        w_k_view = w_mm[:].rearrange("p (k o) -> p k o", o=P)

        # ---- conv slabs ----
        max_xpad_len = (MAX_SO + 2) * WP
        max_stage = (MAX_SO + 2) * W
        out_qs = [nc.gpsimd, nc.scalar]
        slab_idx = 0
        for img in range(IMGS):
            h0 = 0
            for so in SLAB_PLAN[img]:
                slab_in = so + 2
                xpad_len = slab_in * WP
                # +2 tail guard: the last group's (dh=2,dw=2) shifted view
                # reads 2 elements past the padded slab
                xpad = xp.tile([P, max_xpad_len + 2], f16, tag="xpad")
                xpad3 = xpad[:, :xpad_len].rearrange("p (r c) -> p r c", c=WP)
                # zero borders: cols {0,113} every row; pad row at image edge
                nc.gpsimd.memset(xpad3[:, :, 0:114:113], 0.0)
                nc.gpsimd.memset(xpad[:, xpad_len:xpad_len + 2], 0.0)
                if h0 == 0:
                    nc.gpsimd.memset(xpad[:, 0:WP], 0.0)
                elif h0 + so == H:
                    nc.gpsimd.memset(xpad[:, (slab_in - 1) * WP:xpad_len], 0.0)
                # interior rows: f32 staged load, scalar-engine cast to f16
                r_lo = max(0, h0 - 1)
                r_hi = min(H, h0 + so + 1)
                j0 = r_lo - (h0 - 1)
                nrows = r_hi - r_lo
                if img == 0 and h0 == 0:
                    stage = pre_stage
                else:
                    stage = xs.tile([P, max_stage], f32, tag="xstage")
                    nc.sync.dma_start(
                        stage[:, :nrows * W], x_t.ap()[img, :, r_lo:r_hi, :]
                    )
                nc.scalar.copy(
                    xpad3[:, j0:j0 + nrows, 1:1 + W],
                    stage[:, :nrows * W].rearrange("p (r c) -> p r c", c=W),
                )

                # packed 112-wide output slab (junk cols dropped at evac)
                oslab = op.tile([P, MAX_SO * W], f32, tag="oslab")
                for g in range(so // GR):
                    q0 = g * GR * WP
                    n = GR * WP  # 456
                    pst = ps.tile([P, n], f32, tag="pst")
                    for k in range(9):
                        dh, dw = divmod(k, 3)
                        off = q0 + dh * WP + dw
                        nc.tensor.matmul(
                            pst[:, :n],
                            w_k_view[:, k, :],
                            xpad[:, off:off + n],
                            start=(k == 0),
                            stop=(k == 8),
                        )
                    # evac: fused bias add, drop junk cols -> packed rows
                    src = pst[:, :n].rearrange("p (r c) -> p r c", c=WP)[
                        :, :, 0:W
                    ]
                    dst = oslab[:, g * GR * W:(g + 1) * GR * W].rearrange(
                        "p (r c) -> p r c", c=W
                    )
                    nc.vector.tensor_scalar_add(dst, src, bias_s[:, 0:1])

                osrc = oslab[:, :so * W].rearrange("p (r c) -> p r c", c=W)
                out_qs[slab_idx % 2].dma_start(
                    out_t.ap()[img, :, h0:h0 + so, :], osrc
                )
                slab_idx += 1
                h0 += so

    nc.compile()
    return nc


def _make_in_maps(inputs):
    x = np.ascontiguousarray(np.asarray(inputs["x"], dtype=np.float32))
    cent = np.asarray(inputs["centroids"], dtype=np.float32).reshape(512, 9)
    idxT = np.asarray(inputs["idx"]).reshape(P, P).T          # [i, o]
    scalesT = np.ascontiguousarray(
        np.asarray(inputs["scales"], dtype=np.float32).reshape(P, P).T
    )
    cutT = np.ascontiguousarray(
        np.asarray(inputs["cut"], dtype=np.float32).reshape(P, P).T
    )
    bias = np.ascontiguousarray(
        np.asarray(inputs["bias"], dtype=np.float32).reshape(P, 1)
    )
    # k-major: wrawT[i, k, o] = cent[idxT[i, o], k]
    wrawT = np.ascontiguousarray(
        cent[idxT].transpose(0, 2, 1).reshape(P, P * 9)
    )

    base = {"scalesT": scalesT, "cutT": cutT, "bias": bias, "wrawT": wrawT}
    maps = []
    for c in range(N_CORES):
        m = dict(base)
        m["x"] = np.ascontiguousarray(x[IMGS * c:IMGS * (c + 1)])
        maps.append(m)
    return maps


def _get_nc():
    if "nc" not in _CACHE:
        _CACHE["nc"] = _build()
    return _CACHE["nc"]


def _run(inputs, trace=False):
    nc = _get_nc()
    in_maps = _make_in_maps(inputs)
    res = bass_utils.run_bass_kernel_spmd(
        nc, in_maps, core_ids=list(range(N_CORES)), trace=trace
    )
    out = np.concatenate([res.results[c]["out"] for c in range(N_CORES)], axis=0)
    return out, res


def kernel(**inputs) -> np.ndarray:
    out, _ = _run(inputs, trace=False)
    return out


# revision 5
# speedup vs baseline: 1.0030x; 1.0030x over previous
"""VQ-codebook 3x3 conv (nn_CConv) on 8 Trainium2 NeuronCores.

Sharding: data-parallel over the batch (16 images -> 2 per core); the small
codebook-derived weights / scales / bias are replicated to every core.
Host-side work is layout only: batch split, reshape/transpose of the index
and scale matrices, and the codebook row gather (pure indexing, no
arithmetic).

Per-core device program (one NEFF, SPMD over 8 cores):
  - weight build (on device): fp16 round-trip of scales (dequant emulation),
    multiply by cut, broadcast-multiply onto the gathered codebook rows
    (shipped k-major so the multiply is one unit-stride op); weights stored
    k-major so each of the 9 taps is a contiguous [128(in), 128(out)] fp16
    stationary block.  Weight DMAs ride the scalar HWDGE queue so they do
    not serialize behind input loads.
  - conv: each image is zero-padded to rows of width 114 in SBUF (borders
    zeroed on-chip); the 3x3 conv is 9 accumulating PE matmuls over shifted
    views of the flattened padded image, fp16 in / fp32 PSUM accumulate.
    PSUM groups are row-aligned (N = 4 rows x 114 = 456) so the junk
    columns w in {112,113} can be dropped during PSUM->SBUF evacuation
    (fused per-partition bias add on the vector engine).  The output slab
    in SBUF is therefore densely packed 112-wide, which makes the output
    DMA fully contiguous on both sides (large packets, ~2.4x the effective
    DMA bandwidth of per-row 448B descriptors).
  - output DMAs alternate between the gpsimd and vector HWDGE queues;
    input loads keep the sync queue to themselves.
  - images are processed in row-slabs (4/24/28 output rows; tiny slabs at
    the kernel's two ends shorten the serial prologue and the final-DMA
    tail); input loads are double-buffered and cast f32->f16 by the scalar
    engine.
  - 8 dummy warm-up matmuls run during the prologue so the PE HAM clock
    gate reaches 2.4 GHz before the real matmuls start.
"""
import sys
import types
from contextlib import ExitStack

import numpy as np

import concourse.tile as tile
from concourse import bacc, mybir


def _ensure_axon_hooks_module():
    """This image's antenv package lacks axon_hooks; bass_utils imports it
    when tracing is requested (e.g. BASS_TRACE=1). Provide a no-op shim."""
    try:
        import antenv

        if "antenv.axon_hooks" not in sys.modules and not hasattr(
            antenv, "axon_hooks"
        ):
            mod = types.ModuleType("antenv.axon_hooks")
            holder = [None]
            mod.set_axon_ntff_profile_hook = lambda h: holder.__setitem__(0, h)
            mod.get_axon_ntff_profile_hook = lambda: holder[0]
            antenv.axon_hooks = mod
            sys.modules["antenv.axon_hooks"] = mod
    except Exception:
        pass


_ensure_axon_hooks_module()

from concourse import bass_utils  # noqa: E402

P = 128
H = W = 112
WP = 114
IMGS = 2
N_CORES = 8

f32 = mybir.dt.float32
f16 = mybir.dt.float16

SLAB_PLAN = {0: [4, 24, 28, 28, 28], 1: [28, 28, 24, 16, 8, 8]}
MAX_SO = 28
GR = 4           # output rows per PSUM group; N = GR*WP = 456
WARM_MMS = 16

_CACHE = {}


def _build():
    nc = bacc.Bacc("TRN2", target_bir_lowering=False, debug=False)

    x_t = nc.dram_tensor("x", [IMGS, P, H, W], f32, kind="ExternalInput")
    scalesT_t = nc.dram_tensor("scalesT", [P, P], f32, kind="ExternalInput")
    cutT_t = nc.dram_tensor("cutT", [P, P], f32, kind="ExternalInput")
    bias_t = nc.dram_tensor("bias", [P, 1], f32, kind="ExternalInput")
    wrawT_t = nc.dram_tensor("wrawT", [P, 9 * P], f32, kind="ExternalInput")
    out_t = nc.dram_tensor("out", [IMGS, P, H, W], f32, kind="ExternalOutput")

    with tile.TileContext(nc) as tc, ExitStack() as ctx:
        wb = ctx.enter_context(tc.tile_pool(name="wb", bufs=1))
        xp = ctx.enter_context(tc.tile_pool(name="xp", bufs=4))
        op = ctx.enter_context(tc.tile_pool(name="op", bufs=4))
        ps = ctx.enter_context(tc.tile_pool(name="ps", bufs=6, space="PSUM"))
        xs = ctx.enter_context(tc.tile_pool(name="xs", bufs=4))

        with tc.high_priority():
            # PE warmup tile: first gpsimd op so the PE can start ASAP
            wrm = wb.tile([P, 512], f16, tag="warm")
            nc.gpsimd.memset(wrm[:], 0.0)

            # ---- weight DMAs on the scalar queue (parallel to inputs) ----
            w_raw = wb.tile([P, 9 * P], f32, tag="w_raw")
            nc.scalar.dma_start(w_raw[:], wrawT_t.ap())
            sc_in = wb.tile([P, P], f32, tag="sc_in")
            nc.scalar.dma_start(sc_in[:], scalesT_t.ap())
            cut_s = wb.tile([P, P], f32, tag="cut")
            nc.scalar.dma_start(cut_s[:], cutT_t.ap())
            bias_s = wb.tile([P, 1], f32, tag="bias")
            nc.scalar.dma_start(bias_s[:], bias_t.ap())

            # peel slab (0,0) input load so it heads the sync DMA queue
            so0 = SLAB_PLAN[0][0]
            nrows0 = min(H, so0 + 1)
            pre_stage = xs.tile([P, (MAX_SO + 2) * W], f32, tag="xstage")
            nc.sync.dma_start(
                pre_stage[:, :nrows0 * W], x_t.ap()[0, :, 0:nrows0, :]
            )

            # PE warmup: HAM un-throttles to 2.4 GHz during the prologue
            pw = ps.tile([P, 512], f32, tag="pst")
            for _ in range(WARM_MMS):
                nc.tensor.matmul(pw[:], wrm[:, :P], wrm[:], start=True, stop=True)

            # ---- weight build ----
            sc16 = wb.tile([P, P], f16, tag="sc16")
            nc.vector.tensor_copy(sc16[:], sc_in[:])
            sc = wb.tile([P, P], f32, tag="sc")
            nc.vector.tensor_copy(sc[:], sc16[:])
            scc = wb.tile([P, P], f32, tag="scc")
            nc.vector.tensor_tensor(
                out=scc[:], in0=sc[:], in1=cut_s[:], op=mybir.AluOpType.mult
            )

            # w_mm[i, k, o] = w_raw[i, k, o] * scc[i, o]  (w_raw is k-major)
            w_mm = wb.tile([P, 9 * P], f16, tag="w_mm")
            w_raw3 = w_raw[:].rearrange("p (k o) -> p k o", o=P)
            scc3 = scc[:].rearrange(
                "p (one o) -> p one o", one=1
            ).to_broadcast([P, 9, P])
            w_mm3 = w_mm[:].rearrange("p (k o) -> p k o", o=P)
            nc.vector.tensor_tensor(
                out=w_mm3, in0=w_raw3, in1=scc3, op=mybir.AluOpType.mult
            )
        w_k_view = w_mm[:].rearrange("p (k o) -> p k o", o=P)

        # ---- conv slabs ----
        max_xpad_len = (MAX_SO + 2) * WP
        max_stage = (MAX_SO + 2) * W
        out_qs = [nc.gpsimd, nc.scalar]
        slab_idx = 0
        for img in range(IMGS):
            h0 = 0
            for so in SLAB_PLAN[img]:
                slab_in = so + 2
                xpad_len = slab_in * WP
                # +2 tail guard: the last group's (dh=2,dw=2) shifted view
                # reads 2 elements past the padded slab
                xpad = xp.tile([P, max_xpad_len + 2], f16, tag="xpad")
                xpad3 = xpad[:, :xpad_len].rearrange("p (r c) -> p r c", c=WP)
                # zero borders: cols {0,113} every row; pad row at image edge
                nc.gpsimd.memset(xpad3[:, :, 0:114:113], 0.0)
                nc.gpsimd.memset(xpad[:, xpad_len:xpad_len + 2], 0.0)
                if h0 == 0:
                    nc.gpsimd.memset(xpad[:, 0:WP], 0.0)
                elif h0 + so == H:
                    nc.gpsimd.memset(xpad[:, (slab_in - 1) * WP:xpad_len], 0.0)
                # interior rows: f32 staged load, scalar-engine cast to f16
                r_lo = max(0, h0 - 1)
                r_hi = min(H, h0 + so + 1)
                j0 = r_lo - (h0 - 1)
                nrows = r_hi - r_lo
                if img == 0 and h0 == 0:
                    stage = pre_stage
                else:
                    stage = xs.tile([P, max_stage], f32, tag="xstage")
                    nc.sync.dma_start(
                        stage[:, :nrows * W], x_t.ap()[img, :, r_lo:r_hi, :]
                    )
                nc.scalar.copy(
                    xpad3[:, j0:j0 + nrows, 1:1 + W],
                    stage[:, :nrows * W].rearrange("p (r c) -> p r c", c=W),
                )

                # packed 112-wide output slab (junk cols dropped at evac)
                oslab = op.tile([P, MAX_SO * W], f32, tag="oslab")
                for g in range(so // GR):
                    q0 = g * GR * WP
                    n = GR * WP  # 456
                    pst = ps.tile([P, n], f32, tag="pst")
                    for k in range(9):
                        dh, dw = divmod(k, 3)
                        off = q0 + dh * WP + dw
                        nc.tensor.matmul(
                            pst[:, :n],
                            w_k_view[:, k, :],
                            xpad[:, off:off + n],
                            start=(k == 0),
                            stop=(k == 8),
                        )
                    # evac: fused bias add, drop junk cols -> packed rows
                    src = pst[:, :n].rearrange("p (r c) -> p r c", c=WP)[
                        :, :, 0:W
                    ]
                    dst = oslab[:, g * GR * W:(g + 1) * GR * W].rearrange(
                        "p (r c) -> p r c", c=W
                    )
                    nc.vector.tensor_scalar_add(dst, src, bias_s[:, 0:1])

                osrc = oslab[:, :so * W].rearrange("p (r c) -> p r c", c=W)
                out_qs[slab_idx % 2].dma_start(
                    out_t.ap()[img, :, h0:h0 + so, :], osrc
                )
                slab_idx += 1
                h0 += so

    nc.compile()
    return nc


def _make_in_maps(inputs):
    x = np.ascontiguousarray(np.asarray(inputs["x"], dtype=np.float32))
    cent = np.asarray(inputs["centroids"], dtype=np.float32).reshape(512, 9)
    idxT = np.asarray(inputs["idx"]).reshape(P, P).T          # [i, o]
    scalesT = np.ascontiguousarray(
        np.asarray(inputs["scales"], dtype=np.float32).reshape(P, P).T
    )
    cutT = np.ascontiguousarray(
        np.asarray(inputs["cut"], dtype=np.float32).reshape(P, P).T
    )
    bias = np.ascontiguousarray(
        np.asarray(inputs["bias"], dtype=np.float32).reshape(P, 1)
    )
    # k-major: wrawT[i, k, o] = cent[idxT[i, o], k]
    wrawT = np.ascontiguousarray(
        cent[idxT].transpose(0, 2, 1).reshape(P, P * 9)
    )

    base = {"scalesT": scalesT, "cutT": cutT, "bias": bias, "wrawT": wrawT}
    maps = []
    for c in range(N_CORES):
        m = dict(base)
        m["x"] = np.ascontiguousarray(x[IMGS * c:IMGS * (c + 1)])
        maps.append(m)
    return maps


def _get_nc():
    if "nc" not in _CACHE:
        _CACHE["nc"] = _build()
    return _CACHE["nc"]


def _run(inputs, trace=False):
    nc = _get_nc()
    in_maps = _make_in_maps(inputs)
    res = bass_utils.run_bass_kernel_spmd(
        nc, in_maps, core_ids=list(range(N_CORES)), trace=trace
    )
    out = np.concatenate([res.results[c]["out"] for c in range(N_CORES)], axis=0)
    return out, res


def kernel(**inputs) -> np.ndarray:
    out, _ = _run(inputs, trace=False)
    return out


# revision 6
# speedup vs baseline: 1.0407x; 1.0376x over previous
"""VQ-codebook 3x3 conv (nn_CConv) on 8 Trainium2 NeuronCores.

Sharding: data-parallel over the batch (16 images -> 2 per core); the small
codebook-derived weights / scales / bias are replicated to every core.
Host-side work is layout only: batch split, reshape/transpose of the index
and scale matrices, and the codebook row gather (pure indexing, no
arithmetic).

Per-core device program (one NEFF, SPMD over 8 cores):
  - weight build (on device): fp16 round-trip of scales (dequant emulation),
    multiply by cut, broadcast-multiply onto the gathered codebook rows
    (shipped k-major so the multiply is one unit-stride op); weights stored
    k-major so each of the 9 taps is a contiguous [128(in), 128(out)] fp16
    stationary block.  Weight DMAs ride the scalar HWDGE queue so they do
    not serialize behind input loads.
  - conv: each image is zero-padded to rows of width 114 in SBUF (borders
    zeroed on-chip); the 3x3 conv is 9 accumulating PE matmuls over shifted
    views of the flattened padded image, fp16 in / fp32 PSUM accumulate.
    PSUM groups are row-aligned (N = 4 rows x 114 = 456) so the junk
    columns w in {112,113} can be dropped during PSUM->SBUF evacuation
    (fused per-partition bias add on the vector engine).  The output slab
    in SBUF is therefore densely packed 112-wide, which makes the output
    DMA fully contiguous on both sides (large packets, ~2.4x the effective
    DMA bandwidth of per-row 448B descriptors).
  - output DMAs alternate between the gpsimd and vector HWDGE queues;
    input loads keep the sync queue to themselves.
  - images are processed in row-slabs (4/24/28 output rows; tiny slabs at
    the kernel's two ends shorten the serial prologue and the final-DMA
    tail); input loads are double-buffered and cast f32->f16 by the scalar
    engine.
  - 8 dummy warm-up matmuls run during the prologue so the PE HAM clock
    gate reaches 2.4 GHz before the real matmuls start.
"""
import sys
import types
from contextlib import ExitStack

import numpy as np

import concourse.tile as tile
from concourse import bacc, mybir


def _ensure_axon_hooks_module():
    """This image's antenv package lacks axon_hooks; bass_utils imports it
    when tracing is requested (e.g. BASS_TRACE=1). Provide a no-op shim."""
    try:
        import antenv

        if "antenv.axon_hooks" not in sys.modules and not hasattr(
            antenv, "axon_hooks"
        ):
            mod = types.ModuleType("antenv.axon_hooks")
            holder = [None]
            mod.set_axon_ntff_profile_hook = lambda h: holder.__setitem__(0, h)
            mod.get_axon_ntff_profile_hook = lambda: holder[0]
            antenv.axon_hooks = mod
            sys.modules["antenv.axon_hooks"] = mod
    except Exception:
        pass


_ensure_axon_hooks_module()

from concourse import bass_utils  # noqa: E402

P = 128
H = W = 112
WP = 114
IMGS = 2
N_CORES = 8

f32 = mybir.dt.float32
f16 = mybir.dt.float16

SLAB_PLAN = {0: [4, 24, 28, 28, 28], 1: [28, 28, 24, 16, 8, 8]}
MAX_SO = 28
GR = 4           # output rows per PSUM group; N = GR*WP = 456
WARM_MMS = 16

_CACHE = {}


def _build():
    nc = bacc.Bacc("TRN2", target_bir_lowering=False, debug=False)

    x_t = nc.dram_tensor("x", [IMGS, P, H, W], f32, kind="ExternalInput")
    scalesT_t = nc.dram_tensor("scalesT", [P, P], f32, kind="ExternalInput")
    cutT_t = nc.dram_tensor("cutT", [P, P], f32, kind="ExternalInput")
    bias_t = nc.dram_tensor("bias", [P, 1], f32, kind="ExternalInput")
    wrawT_t = nc.dram_tensor("wrawT", [P, 9 * P], f32, kind="ExternalInput")
    out_t = nc.dram_tensor("out", [IMGS, P, H, W], f32, kind="ExternalOutput")

    with tile.TileContext(nc) as tc, ExitStack() as ctx:
        wb = ctx.enter_context(tc.tile_pool(name="wb", bufs=1))
        xp = ctx.enter_context(tc.tile_pool(name="xp", bufs=4))
        op = ctx.enter_context(tc.tile_pool(name="op", bufs=4))
        ps = ctx.enter_context(tc.tile_pool(name="ps", bufs=6, space="PSUM"))
        xs = ctx.enter_context(tc.tile_pool(name="xs", bufs=4))

        with tc.high_priority():
            # PE warmup tile: first gpsimd op so the PE can start ASAP
            wrm = wb.tile([P, 512], f16, tag="warm")
            nc.gpsimd.memset(wrm[:], 0.0)

            # ---- weight DMAs head the sync queue, small-first, so they
            # cannot be starved by large input descriptors on another queue
            sc_in = wb.tile([P, P], f32, tag="sc_in")
            nc.sync.dma_start(sc_in[:], scalesT_t.ap())
            cut_s = wb.tile([P, P], f32, tag="cut")
            nc.sync.dma_start(cut_s[:], cutT_t.ap())
            bias_s = wb.tile([P, 1], f32, tag="bias")
            nc.sync.dma_start(bias_s[:], bias_t.ap())
            w_raw = wb.tile([P, 9 * P], f32, tag="w_raw")
            nc.sync.dma_start(w_raw[:], wrawT_t.ap())

            # peel slab (0,0) input load right behind the weights
            so0 = SLAB_PLAN[0][0]
            nrows0 = min(H, so0 + 1)
            pre_stage = xs.tile([P, (MAX_SO + 2) * W], f32, tag="xstage")
            nc.sync.dma_start(
                pre_stage[:, :nrows0 * W], x_t.ap()[0, :, 0:nrows0, :]
            )

            # PE warmup: HAM un-throttles to 2.4 GHz during the prologue
            pw = ps.tile([P, 512], f32, tag="pst")
            for _ in range(WARM_MMS):
                nc.tensor.matmul(pw[:], wrm[:, :P], wrm[:], start=True, stop=True)

            # ---- weight build ----
            sc16 = wb.tile([P, P], f16, tag="sc16")
            nc.vector.tensor_copy(sc16[:], sc_in[:])
            sc = wb.tile([P, P], f32, tag="sc")
            nc.vector.tensor_copy(sc[:], sc16[:])
            scc = wb.tile([P, P], f32, tag="scc")
            nc.vector.tensor_tensor(
                out=scc[:], in0=sc[:], in1=cut_s[:], op=mybir.AluOpType.mult
            )

            # w_mm[i, k, o] = w_raw[i, k, o] * scc[i, o]  (w_raw is k-major)
            w_mm = wb.tile([P, 9 * P], f16, tag="w_mm")
            w_raw3 = w_raw[:].rearrange("p (k o) -> p k o", o=P)
            scc3 = scc[:].rearrange(
                "p (one o) -> p one o", one=1
            ).to_broadcast([P, 9, P])
            w_mm3 = w_mm[:].rearrange("p (k o) -> p k o", o=P)
            nc.vector.tensor_tensor(
                out=w_mm3, in0=w_raw3, in1=scc3, op=mybir.AluOpType.mult
            )
        w_k_view = w_mm[:].rearrange("p (k o) -> p k o", o=P)

        # ---- conv slabs ----
        max_xpad_len = (MAX_SO + 2) * WP
        max_stage = (MAX_SO + 2) * W
        out_qs = [nc.gpsimd, nc.scalar]
        slab_idx = 0
        for img in range(IMGS):
            h0 = 0
            for so in SLAB_PLAN[img]:
                slab_in = so + 2
                xpad_len = slab_in * WP
                # +2 tail guard: the last group's (dh=2,dw=2) shifted view
                # reads 2 elements past the padded slab
                xpad = xp.tile([P, max_xpad_len + 2], f16, tag="xpad")
                xpad3 = xpad[:, :xpad_len].rearrange("p (r c) -> p r c", c=WP)
                # zero borders: cols {0,113} every row; pad row at image edge
                nc.gpsimd.memset(xpad3[:, :, 0:114:113], 0.0)
                nc.gpsimd.memset(xpad[:, xpad_len:xpad_len + 2], 0.0)
                if h0 == 0:
                    nc.gpsimd.memset(xpad[:, 0:WP], 0.0)
                elif h0 + so == H:
                    nc.gpsimd.memset(xpad[:, (slab_in - 1) * WP:xpad_len], 0.0)
                # interior rows: f32 staged load, scalar-engine cast to f16
                r_lo = max(0, h0 - 1)
                r_hi = min(H, h0 + so + 1)
                j0 = r_lo - (h0 - 1)
                nrows = r_hi - r_lo
                if img == 0 and h0 == 0:
                    stage = pre_stage
                else:
                    stage = xs.tile([P, max_stage], f32, tag="xstage")
                    nc.sync.dma_start(
                        stage[:, :nrows * W], x_t.ap()[img, :, r_lo:r_hi, :]
                    )
                nc.scalar.copy(
                    xpad3[:, j0:j0 + nrows, 1:1 + W],
                    stage[:, :nrows * W].rearrange("p (r c) -> p r c", c=W),
                )

                # packed 112-wide output slab (junk cols dropped at evac)
                oslab = op.tile([P, MAX_SO * W], f32, tag="oslab")
                for g in range(so // GR):
                    q0 = g * GR * WP
                    n = GR * WP  # 456
                    pst = ps.tile([P, n], f32, tag="pst")
                    for k in range(9):
                        dh, dw = divmod(k, 3)
                        off = q0 + dh * WP + dw
                        nc.tensor.matmul(
                            pst[:, :n],
                            w_k_view[:, k, :],
                            xpad[:, off:off + n],
                            start=(k == 0),
                            stop=(k == 8),
                        )
                    # evac: fused bias add, drop junk cols -> packed rows
                    src = pst[:, :n].rearrange("p (r c) -> p r c", c=WP)[
                        :, :, 0:W
                    ]
                    dst = oslab[:, g * GR * W:(g + 1) * GR * W].rearrange(
                        "p (r c) -> p r c", c=W
                    )
                    nc.vector.tensor_scalar_add(dst, src, bias_s[:, 0:1])

                osrc = oslab[:, :so * W].rearrange("p (r c) -> p r c", c=W)
                out_qs[slab_idx % 2].dma_start(
                    out_t.ap()[img, :, h0:h0 + so, :], osrc
                )
                slab_idx += 1
                h0 += so

    nc.compile()
    return nc


def _make_in_maps(inputs):
    x = np.ascontiguousarray(np.asarray(inputs["x"], dtype=np.float32))
    cent = np.asarray(inputs["centroids"], dtype=np.float32).reshape(512, 9)
    idxT = np.asarray(inputs["idx"]).reshape(P, P).T          # [i, o]
    scalesT = np.ascontiguousarray(
        np.asarray(inputs["scales"], dtype=np.float32).reshape(P, P).T
    )
    cutT = np.ascontiguousarray(
        np.asarray(inputs["cut"], dtype=np.float32).reshape(P, P).T
    )
    bias = np.ascontiguousarray(
        np.asarray(inputs["bias"], dtype=np.float32).reshape(P, 1)
    )
    # k-major: wrawT[i, k, o] = cent[idxT[i, o], k]
    wrawT = np.ascontiguousarray(
        cent[idxT].transpose(0, 2, 1).reshape(P, P * 9)
    )

    base = {"scalesT": scalesT, "cutT": cutT, "bias": bias, "wrawT": wrawT}
    maps = []
    for c in range(N_CORES):
        m = dict(base)
        m["x"] = np.ascontiguousarray(x[IMGS * c:IMGS * (c + 1)])
        maps.append(m)
    return maps


def _get_nc():
    if "nc" not in _CACHE:
        _CACHE["nc"] = _build()
    return _CACHE["nc"]


def _run(inputs, trace=False):
    nc = _get_nc()
    in_maps = _make_in_maps(inputs)
    res = bass_utils.run_bass_kernel_spmd(
        nc, in_maps, core_ids=list(range(N_CORES)), trace=trace
    )
    out = np.concatenate([res.results[c]["out"] for c in range(N_CORES)], axis=0)
    return out, res


def kernel(**inputs) -> np.ndarray:
    out, _ = _run(inputs, trace=False)
    return out


# revision 11
# speedup vs baseline: 1.0805x; 1.0382x over previous
"""VQ-codebook 3x3 conv (nn_CConv) on 8 Trainium2 NeuronCores.

Sharding: data-parallel over the batch (16 images -> 2 per core); the small
codebook-derived weights / scales / bias are replicated to every core.
Host-side work is layout only: batch split, reshape/transpose of the index
and scale matrices, and the codebook row gather (pure indexing, no
arithmetic).

Per-core device program (one NEFF, SPMD over 8 cores):
  - weight build (on device): fp16 round-trip of scales (dequant emulation),
    multiply by cut, broadcast-multiply onto the gathered codebook rows
    (shipped k-major so the multiply is one unit-stride op); weights stored
    k-major so each of the 9 taps is a contiguous [128(in), 128(out)] fp16
    stationary block.  Weight DMAs ride the scalar HWDGE queue so they do
    not serialize behind input loads.
  - conv: each image is zero-padded to rows of width 114 in SBUF (borders
    zeroed on-chip); the 3x3 conv is 9 accumulating PE matmuls over shifted
    views of the flattened padded image, fp16 in / fp32 PSUM accumulate.
    PSUM groups are row-aligned (N = 4 rows x 114 = 456) so the junk
    columns w in {112,113} can be dropped during PSUM->SBUF evacuation
    (fused per-partition bias add on the vector engine).  The output slab
    in SBUF is therefore densely packed 112-wide, which makes the output
    DMA fully contiguous on both sides (large packets, ~2.4x the effective
    DMA bandwidth of per-row 448B descriptors).
  - output DMAs alternate between the gpsimd and vector HWDGE queues;
    input loads keep the sync queue to themselves.
  - images are processed in row-slabs (4/24/28 output rows; tiny slabs at
    the kernel's two ends shorten the serial prologue and the final-DMA
    tail); input loads are double-buffered and cast f32->f16 by the scalar
    engine.
  - 8 dummy warm-up matmuls run during the prologue so the PE HAM clock
    gate reaches 2.4 GHz before the real matmuls start.
"""
import sys
import types
from contextlib import ExitStack

import numpy as np

import concourse.tile as tile
from concourse import bacc, mybir


def _ensure_axon_hooks_module():
    """This image's antenv package lacks axon_hooks; bass_utils imports it
    when tracing is requested (e.g. BASS_TRACE=1). Provide a no-op shim."""
    try:
        import antenv

        if "antenv.axon_hooks" not in sys.modules and not hasattr(
            antenv, "axon_hooks"
        ):
            mod = types.ModuleType("antenv.axon_hooks")
            holder = [None]
            mod.set_axon_ntff_profile_hook = lambda h: holder.__setitem__(0, h)
            mod.get_axon_ntff_profile_hook = lambda: holder[0]
            antenv.axon_hooks = mod
            sys.modules["antenv.axon_hooks"] = mod
    except Exception:
        pass


_ensure_axon_hooks_module()

from concourse import bass_utils  # noqa: E402

P = 128
H = W = 112
WP = 114
IMGS = 2
N_CORES = 8

f32 = mybir.dt.float32
f16 = mybir.dt.float16

SLAB_PLAN = {0: [4, 8, 16, 28, 28, 28], 1: [28, 28, 24, 16, 12, 4]}
MAX_SO = 28
GR = 4           # output rows per PSUM group; N = GR*WP = 456
WARM_MMS = 16

_CACHE = {}


def _build():
    nc = bacc.Bacc("TRN2", target_bir_lowering=False, debug=False)

    x_t = nc.dram_tensor("x", [IMGS, P, H, W], f32, kind="ExternalInput")
    # wblob = [scalesT | cutT | bias | wrawT] concatenated along the free dim
    # so the whole weight payload is ONE DMA with 5.6KB per-partition packets
    # (small separate 512B-packet DMAs get starved by the input queue's big
    # descriptors in the DMA engines' per-packet round-robin).
    wblob_t = nc.dram_tensor(
        "wblob", [P, 2 * P + 1 + 9 * P], f32, kind="ExternalInput"
    )
    out_t = nc.dram_tensor("out", [IMGS, P, H, W], f32, kind="ExternalOutput")

    with tile.TileContext(nc) as tc, ExitStack() as ctx:
        wb = ctx.enter_context(tc.tile_pool(name="wb", bufs=1))
        xp = ctx.enter_context(tc.tile_pool(name="xp", bufs=4))
        op = ctx.enter_context(tc.tile_pool(name="op", bufs=4))
        ps = ctx.enter_context(tc.tile_pool(name="ps", bufs=6, space="PSUM"))
        xs = ctx.enter_context(tc.tile_pool(name="xs", bufs=4))

        with tc.high_priority():
            # PE warmup tile: first vector op so the PE can start ASAP
            wrm = wb.tile([P, 512], f16, tag="warm")
            nc.vector.memset(wrm[:], 0.0)

            # warm the scalar engine's activation table (data-independent
            # ~2.7us ACT_TABLE_LOAD) before any real cast needs it
            dmy = wb.tile([P, 8], f16, tag="dmy")
            nc.scalar.copy(dmy[:], wrm[:, 0:8])

            # ---- single packed weight DMA at the head of the sync queue
            wblob = wb.tile([P, 2 * P + 1 + 9 * P], f32, tag="wblob")
            nc.sync.dma_start(wblob[:], wblob_t.ap())
            sc_in = wblob[:, 0:P]
            cut_s = wblob[:, P:2 * P]
            bias_s = wblob[:, 2 * P:2 * P + 1]
            w_raw = wblob[:, 2 * P + 1:]

            # peel slab (0,0) input load right behind the weights
            so0 = SLAB_PLAN[0][0]
            nrows0 = min(H, so0 + 1)
            pre_stage = xs.tile([P, (MAX_SO + 2) * W], f32, tag="xstage")
            nc.sync.dma_start(
                pre_stage[:, :nrows0 * W], x_t.ap()[0, :, 0:nrows0, :]
            )

            # PE warmup: HAM un-throttles to 2.4 GHz during the prologue
            pw = ps.tile([P, 512], f32, tag="pst")
            for _ in range(WARM_MMS):
                nc.tensor.matmul(pw[:], wrm[:, :P], wrm[:], start=True, stop=True)

            # ---- weight build ----
            sc16 = wb.tile([P, P], f16, tag="sc16")
            nc.vector.tensor_copy(sc16[:], sc_in)
            sc = wb.tile([P, P], f32, tag="sc")
            nc.vector.tensor_copy(sc[:], sc16[:])
            scc = wb.tile([P, P], f32, tag="scc")
            nc.vector.tensor_tensor(
                out=scc[:], in0=sc[:], in1=cut_s, op=mybir.AluOpType.mult
            )

            # w_mm[i, k, o] = w_raw[i, k, o] * scc[i, o]  (w_raw is k-major)
            w_mm = wb.tile([P, 9 * P], f16, tag="w_mm")
            w_raw3 = w_raw.rearrange("p (k o) -> p k o", o=P)
            scc3 = scc[:].rearrange(
                "p (one o) -> p one o", one=1
            ).to_broadcast([P, 9, P])
            w_mm3 = w_mm[:].rearrange("p (k o) -> p k o", o=P)
            nc.vector.tensor_tensor(
                out=w_mm3, in0=w_raw3, in1=scc3, op=mybir.AluOpType.mult
            )
        w_k_view = w_mm[:].rearrange("p (k o) -> p k o", o=P)

        # ---- conv slabs ----
        max_xpad_len = (MAX_SO + 2) * WP
        max_stage = (MAX_SO + 2) * W
        out_qs = [nc.gpsimd, nc.scalar]
        slab_idx = 0
        for img in range(IMGS):
            h0 = 0
            for so in SLAB_PLAN[img]:
                slab_in = so + 2
                xpad_len = slab_in * WP
                # +2 tail guard: the last group's (dh=2,dw=2) shifted view
                # reads 2 elements past the padded slab
                xpad = xp.tile([P, max_xpad_len + 2], f16, tag="xpad")
                xpad3 = xpad[:, :xpad_len].rearrange("p (r c) -> p r c", c=WP)
                # zero borders: cols {0,113} every row; pad row at image edge
                nc.gpsimd.memset(xpad3[:, :, 0:114:113], 0.0)
                nc.gpsimd.memset(xpad[:, xpad_len:xpad_len + 2], 0.0)
                if h0 == 0:
                    nc.gpsimd.memset(xpad[:, 0:WP], 0.0)
                elif h0 + so == H:
                    nc.gpsimd.memset(xpad[:, (slab_in - 1) * WP:xpad_len], 0.0)
                # interior rows: f32 staged load, scalar-engine cast to f16
                r_lo = max(0, h0 - 1)
                r_hi = min(H, h0 + so + 1)
                j0 = r_lo - (h0 - 1)
                nrows = r_hi - r_lo
                if img == 0 and h0 == 0:
                    stage = pre_stage
                else:
                    stage = xs.tile([P, max_stage], f32, tag="xstage")
                    nc.sync.dma_start(
                        stage[:, :nrows * W], x_t.ap()[img, :, r_lo:r_hi, :]
                    )
                nc.scalar.copy(
                    xpad3[:, j0:j0 + nrows, 1:1 + W],
                    stage[:, :nrows * W].rearrange("p (r c) -> p r c", c=W),
                )

                # packed 112-wide output slab (junk cols dropped at evac)
                oslab = op.tile([P, MAX_SO * W], f32, tag="oslab")
                for g in range(so // GR):
                    q0 = g * GR * WP
                    n = GR * WP  # 456
                    pst = ps.tile([P, n], f32, tag="pst")
                    for k in range(9):
                        dh, dw = divmod(k, 3)
                        off = q0 + dh * WP + dw
                        nc.tensor.matmul(
                            pst[:, :n],
                            w_k_view[:, k, :],
                            xpad[:, off:off + n],
                            start=(k == 0),
                            stop=(k == 8),
                        )
                    # evac: fused bias add, drop junk cols -> packed rows
                    src = pst[:, :n].rearrange("p (r c) -> p r c", c=WP)[
                        :, :, 0:W
                    ]
                    dst = oslab[:, g * GR * W:(g + 1) * GR * W].rearrange(
                        "p (r c) -> p r c", c=W
                    )
                    nc.vector.tensor_scalar_add(dst, src, bias_s)

                osrc = oslab[:, :so * W].rearrange("p (r c) -> p r c", c=W)
                out_qs[slab_idx % 2].dma_start(
                    out_t.ap()[img, :, h0:h0 + so, :], osrc
                )
                slab_idx += 1
                h0 += so

    nc.compile()
    return nc


def _make_in_maps(inputs):
    x = np.ascontiguousarray(np.asarray(inputs["x"], dtype=np.float32))
    cent = np.asarray(inputs["centroids"], dtype=np.float32).reshape(512, 9)
    idxT = np.asarray(inputs["idx"]).reshape(P, P).T          # [i, o]
    scalesT = np.ascontiguousarray(
        np.asarray(inputs["scales"], dtype=np.float32).reshape(P, P).T
    )
    cutT = np.ascontiguousarray(
        np.asarray(inputs["cut"], dtype=np.float32).reshape(P, P).T
    )
    bias = np.ascontiguousarray(
        np.asarray(inputs["bias"], dtype=np.float32).reshape(P, 1)
    )
    # k-major: wrawT[i, k, o] = cent[idxT[i, o], k]
    wrawT = cent[idxT].transpose(0, 2, 1).reshape(P, P * 9)
    # single packed blob: [scalesT | cutT | bias | wrawT]
    wblob = np.ascontiguousarray(
        np.concatenate([scalesT, cutT, bias, wrawT], axis=1)
    )

    base = {"wblob": wblob}
    maps = []
    for c in range(N_CORES):
        m = dict(base)
        m["x"] = np.ascontiguousarray(x[IMGS * c:IMGS * (c + 1)])
        maps.append(m)
    return maps


def _get_nc():
    if "nc" not in _CACHE:
        _CACHE["nc"] = _build()
    return _CACHE["nc"]


def _run(inputs, trace=False):
    nc = _get_nc()
    in_maps = _make_in_maps(inputs)
    res = bass_utils.run_bass_kernel_spmd(
        nc, in_maps, core_ids=list(range(N_CORES)), trace=trace
    )
    out = np.concatenate([res.results[c]["out"] for c in range(N_CORES)], axis=0)
    return out, res


def kernel(**inputs) -> np.ndarray:
    out, _ = _run(inputs, trace=False)
    return out


# revision 14
# speedup vs baseline: 1.0954x; 1.0138x over previous
"""VQ-codebook 3x3 conv (nn_CConv) on 8 Trainium2 NeuronCores.

Sharding: data-parallel over the batch (16 images -> 2 per core); the small
codebook-derived weights / scales / bias are replicated to every core.
Host-side work is layout only: batch split, reshape/transpose of the index
and scale matrices, and the codebook row gather (pure indexing, no
arithmetic).

Per-core device program (one NEFF, SPMD over 8 cores):
  - weight build (on device): fp16 round-trip of scales (dequant emulation),
    multiply by cut, broadcast-multiply onto the gathered codebook rows
    (shipped k-major so the multiply is one unit-stride op); weights stored
    k-major so each of the 9 taps is a contiguous [128(in), 128(out)] fp16
    stationary block.  Weight DMAs ride the scalar HWDGE queue so they do
    not serialize behind input loads.
  - conv: each image is zero-padded to rows of width 114 in SBUF (borders
    zeroed on-chip); the 3x3 conv is 9 accumulating PE matmuls over shifted
    views of the flattened padded image, fp16 in / fp32 PSUM accumulate.
    PSUM groups are row-aligned (N = 4 rows x 114 = 456) so the junk
    columns w in {112,113} can be dropped during PSUM->SBUF evacuation
    (fused per-partition bias add on the vector engine).  The output slab
    in SBUF is therefore densely packed 112-wide, which makes the output
    DMA fully contiguous on both sides (large packets, ~2.4x the effective
    DMA bandwidth of per-row 448B descriptors).
  - output DMAs alternate between the gpsimd and vector HWDGE queues;
    input loads keep the sync queue to themselves.
  - images are processed in row-slabs (4/24/28 output rows; tiny slabs at
    the kernel's two ends shorten the serial prologue and the final-DMA
    tail); input loads are double-buffered and cast f32->f16 by the scalar
    engine.
  - 8 dummy warm-up matmuls run during the prologue so the PE HAM clock
    gate reaches 2.4 GHz before the real matmuls start.
"""
import sys
import types
from contextlib import ExitStack

import numpy as np

import concourse.tile as tile
from concourse import bacc, mybir


def _ensure_axon_hooks_module():
    """This image's antenv package lacks axon_hooks; bass_utils imports it
    when tracing is requested (e.g. BASS_TRACE=1). Provide a no-op shim."""
    try:
        import antenv

        if "antenv.axon_hooks" not in sys.modules and not hasattr(
            antenv, "axon_hooks"
        ):
            mod = types.ModuleType("antenv.axon_hooks")
            holder = [None]
            mod.set_axon_ntff_profile_hook = lambda h: holder.__setitem__(0, h)
            mod.get_axon_ntff_profile_hook = lambda: holder[0]
            antenv.axon_hooks = mod
            sys.modules["antenv.axon_hooks"] = mod
    except Exception:
        pass


_ensure_axon_hooks_module()

from concourse import bass_utils  # noqa: E402

P = 128
H = W = 112
WP = 114
IMGS = 2
N_CORES = 8

f32 = mybir.dt.float32
f16 = mybir.dt.float16

SLAB_PLAN = {0: [4, 8, 16, 28, 28, 28], 1: [28, 28, 24, 16, 12, 4]}
MAX_SO = 28
GR = 4           # output rows per PSUM group; N = GR*WP = 456
WARM_MMS = 14

_CACHE = {}


def _build():
    nc = bacc.Bacc("TRN2", target_bir_lowering=False, debug=False)

    x_t = nc.dram_tensor("x", [IMGS, P, H, W], f32, kind="ExternalInput")
    # wblob = [scalesT | cutT | bias | wrawT] concatenated along the free dim
    # so the whole weight payload is ONE DMA with 5.6KB per-partition packets
    # (small separate 512B-packet DMAs get starved by the input queue's big
    # descriptors in the DMA engines' per-packet round-robin).
    wblob_t = nc.dram_tensor(
        "wblob", [P, 2 * P + 1 + 9 * P], f32, kind="ExternalInput"
    )
    out_t = nc.dram_tensor("out", [IMGS, P, H, W], f32, kind="ExternalOutput")

    with tile.TileContext(nc) as tc, ExitStack() as ctx:
        wb = ctx.enter_context(tc.tile_pool(name="wb", bufs=1))
        xp = ctx.enter_context(tc.tile_pool(name="xp", bufs=4))
        op = ctx.enter_context(tc.tile_pool(name="op", bufs=4))
        ps = ctx.enter_context(tc.tile_pool(name="ps", bufs=6, space="PSUM"))
        xs = ctx.enter_context(tc.tile_pool(name="xs", bufs=4))

        with tc.high_priority():
            # PE warmup tile: first vector op so the PE can start ASAP
            wrm = wb.tile([P, 512], f16, tag="warm")
            nc.vector.memset(wrm[:], 0.0)

            # warm the scalar engine's activation table (data-independent
            # ~2.7us ACT_TABLE_LOAD) before any real cast needs it
            dmy = wb.tile([P, 8], f16, tag="dmy")
            nc.scalar.copy(dmy[:], wrm[:, 0:8])

            # ---- single packed weight DMA at the head of the sync queue
            wblob = wb.tile([P, 2 * P + 1 + 9 * P], f32, tag="wblob")
            nc.sync.dma_start(wblob[:], wblob_t.ap())
            sc_in = wblob[:, 0:P]
            cut_s = wblob[:, P:2 * P]
            bias_s = wblob[:, 2 * P:2 * P + 1]
            w_raw = wblob[:, 2 * P + 1:]

            # 1-packet dummy DMA: the HWDGE completion a consumer observes
            # lags by one descriptor, so chase each load with a tiny dummy
            # to release its consumers immediately
            dmy_dma = wb.tile([1, 1], f32, tag="dmy_dma")
            nc.sync.dma_start(dmy_dma[:], wblob_t.ap()[0:1, 0:1])

            # peel slab (0,0) input load right behind the weights
            so0 = SLAB_PLAN[0][0]
            nrows0 = min(H, so0 + 1)
            pre_stage = xs.tile([P, (MAX_SO + 2) * W], f32, tag="xstage")
            nc.sync.dma_start(
                pre_stage[:, :nrows0 * W], x_t.ap()[0, :, 0:nrows0, :]
            )
            nc.sync.dma_start(dmy_dma[:], wblob_t.ap()[0:1, 0:1])

            # PE warmup: HAM un-throttles to 2.4 GHz during the prologue
            pw = ps.tile([P, 512], f32, tag="pst")
            for _ in range(WARM_MMS):
                nc.tensor.matmul(pw[:], wrm[:, :P], wrm[:], start=True, stop=True)

            # ---- weight build ----
            sc16 = wb.tile([P, P], f16, tag="sc16")
            nc.vector.tensor_copy(sc16[:], sc_in)
            sc = wb.tile([P, P], f32, tag="sc")
            nc.vector.tensor_copy(sc[:], sc16[:])
            scc = wb.tile([P, P], f32, tag="scc")
            nc.vector.tensor_tensor(
                out=scc[:], in0=sc[:], in1=cut_s, op=mybir.AluOpType.mult
            )

            # w_mm[i, k, o] = w_raw[i, k, o] * scc[i, o]  (w_raw is k-major)
            w_mm = wb.tile([P, 9 * P], f16, tag="w_mm")
            w_raw3 = w_raw.rearrange("p (k o) -> p k o", o=P)
            scc3 = scc[:].rearrange(
                "p (one o) -> p one o", one=1
            ).to_broadcast([P, 9, P])
            w_mm3 = w_mm[:].rearrange("p (k o) -> p k o", o=P)
            nc.vector.tensor_tensor(
                out=w_mm3, in0=w_raw3, in1=scc3, op=mybir.AluOpType.mult
            )
        w_k_view = w_mm[:].rearrange("p (k o) -> p k o", o=P)

        # ---- conv slabs ----
        max_xpad_len = (MAX_SO + 2) * WP
        max_stage = (MAX_SO + 2) * W
        out_qs = [nc.gpsimd, nc.scalar]
        slab_idx = 0
        for img in range(IMGS):
            h0 = 0
            for so in SLAB_PLAN[img]:
                slab_in = so + 2
                xpad_len = slab_in * WP
                # +2 tail guard: the last group's (dh=2,dw=2) shifted view
                # reads 2 elements past the padded slab
                xpad = xp.tile([P, max_xpad_len + 2], f16, tag="xpad")
                xpad3 = xpad[:, :xpad_len].rearrange("p (r c) -> p r c", c=WP)
                # zero borders: cols {0,113} every row; pad row at image edge
                nc.gpsimd.memset(xpad3[:, :, 0:114:113], 0.0)
                nc.gpsimd.memset(xpad[:, xpad_len:xpad_len + 2], 0.0)
                if h0 == 0:
                    nc.gpsimd.memset(xpad[:, 0:WP], 0.0)
                elif h0 + so == H:
                    nc.gpsimd.memset(xpad[:, (slab_in - 1) * WP:xpad_len], 0.0)
                # interior rows: f32 staged load, scalar-engine cast to f16
                r_lo = max(0, h0 - 1)
                r_hi = min(H, h0 + so + 1)
                j0 = r_lo - (h0 - 1)
                nrows = r_hi - r_lo
                if img == 0 and h0 == 0:
                    stage = pre_stage
                else:
                    stage = xs.tile([P, max_stage], f32, tag="xstage")
                    nc.sync.dma_start(
                        stage[:, :nrows * W], x_t.ap()[img, :, r_lo:r_hi, :]
                    )
                    nc.sync.dma_start(dmy_dma[:], wblob_t.ap()[0:1, 0:1])
                nc.scalar.copy(
                    xpad3[:, j0:j0 + nrows, 1:1 + W],
                    stage[:, :nrows * W].rearrange("p (r c) -> p r c", c=W),
                )

                # packed 112-wide output slab (junk cols dropped at evac)
                oslab = op.tile([P, MAX_SO * W], f32, tag="oslab")
                for g in range(so // GR):
                    q0 = g * GR * WP
                    n = GR * WP  # 456
                    pst = ps.tile([P, n], f32, tag="pst")
                    for k in range(9):
                        dh, dw = divmod(k, 3)
                        off = q0 + dh * WP + dw
                        nc.tensor.matmul(
                            pst[:, :n],
                            w_k_view[:, k, :],
                            xpad[:, off:off + n],
                            start=(k == 0),
                            stop=(k == 8),
                        )
                    # evac: fused bias add, drop junk cols -> packed rows
                    src = pst[:, :n].rearrange("p (r c) -> p r c", c=WP)[
                        :, :, 0:W
                    ]
                    dst = oslab[:, g * GR * W:(g + 1) * GR * W].rearrange(
                        "p (r c) -> p r c", c=W
                    )
                    nc.vector.tensor_scalar_add(dst, src, bias_s)

                osrc = oslab[:, :so * W].rearrange("p (r c) -> p r c", c=W)
                out_qs[slab_idx % 2].dma_start(
                    out_t.ap()[img, :, h0:h0 + so, :], osrc
                )
                slab_idx += 1
                h0 += so

    nc.compile()
    return nc


def _make_in_maps(inputs):
    x = np.ascontiguousarray(np.asarray(inputs["x"], dtype=np.float32))
    cent = np.asarray(inputs["centroids"], dtype=np.float32).reshape(512, 9)
    idxT = np.asarray(inputs["idx"]).reshape(P, P).T          # [i, o]
    scalesT = np.ascontiguousarray(
        np.asarray(inputs["scales"], dtype=np.float32).reshape(P, P).T
    )
    cutT = np.ascontiguousarray(
        np.asarray(inputs["cut"], dtype=np.float32).reshape(P, P).T
    )
    bias = np.ascontiguousarray(
        np.asarray(inputs["bias"], dtype=np.float32).reshape(P, 1)
    )
    # k-major: wrawT[i, k, o] = cent[idxT[i, o], k]
    wrawT = cent[idxT].transpose(0, 2, 1).reshape(P, P * 9)
    # single packed blob: [scalesT | cutT | bias | wrawT]
    wblob = np.ascontiguousarray(
        np.concatenate([scalesT, cutT, bias, wrawT], axis=1)
    )

    base = {"wblob": wblob}
    maps = []
    for c in range(N_CORES):
        m = dict(base)
        m["x"] = np.ascontiguousarray(x[IMGS * c:IMGS * (c + 1)])
        maps.append(m)
    return maps


def _get_nc():
    if "nc" not in _CACHE:
        _CACHE["nc"] = _build()
    return _CACHE["nc"]


def _run(inputs, trace=False):
    nc = _get_nc()
    in_maps = _make_in_maps(inputs)
    res = bass_utils.run_bass_kernel_spmd(
        nc, in_maps, core_ids=list(range(N_CORES)), trace=trace
    )
    out = np.concatenate([res.results[c]["out"] for c in range(N_CORES)], axis=0)
    return out, res


def kernel(**inputs) -> np.ndarray:
    out, _ = _run(inputs, trace=False)
    return out
